# revision 24
# baseline (speedup 1.0000x reference)
"""AttentionBlock (GroupNorm + single-head self-attention + proj + residual)
for x[4, 256, 64, 64] on 8 Trainium2 NeuronCores.

Sharding: data-parallel over batch (4) x sequence-parallel over the q/hw dim
(2 halves) = 8 shards, one per core. Each core receives its batch's full
x (rolled so its q-half sits at columns 0:2048), computes GroupNorm + full
K/V + its half of Q, then attention over all 4096 keys for its 2048 queries,
projection and residual. No collectives; host assembles the 8 output shards.

All matmuls run in float32r (full PE rate at N>=256, ~1e-4 relative error).
Attention is computed in "scores-transposed" layout sT[k, q] so that the
P@V contraction consumes exp(sT) directly without any transposes. Softmax
runs without max-subtraction (scores/16 ~ N(0,1), far from overflow); the
denominator accumulates on VectorE and one ones-matmul reduces it across
partitions, with the reciprocal broadcast back via a K=1 matmul.
"""

import numpy as np

import concourse.bass as bass
import concourse.bacc as bacc
import concourse.tile as tile
from concourse import mybir

F32 = mybir.dt.float32
F32R = mybir.dt.float32r

N_CORES = 8
B, C, HH, WW = 4, 256, 64, 64
N = HH * WW            # 4096 pixels (keys)
NH = N // 2            # 2048 queries per core
CT = C // 128          # 2 channel tiles
CH = 512               # free-dim chunk for projections / q-chunks
NCH = N // CH          # 8 chunks over all pixels
QC = NH // CH          # 4 q-chunks
KT = N // 128          # 32 key tiles
GROUPS = 16
GSIZE = C // GROUPS    # 16 channels per group
EPS = 1e-5
SCALE = 1.0 / np.sqrt(np.float32(C))   # 1/16


def _trace_kernel(nc, reps=1):
    xf_ap = nc.dram_tensor("xf", [C, N], F32, kind="ExternalInput").ap()
    wqkvT_ap = nc.dram_tensor("wqkvT", [C, 3 * C], F32, kind="ExternalInput").ap()
    bqkv_ap = nc.dram_tensor("bqkv", [3 * C], F32, kind="ExternalInput").ap()
    wpT_ap = nc.dram_tensor("wpT", [C, C], F32, kind="ExternalInput").ap()
    cb_ap = nc.dram_tensor("cb", [C], F32, kind="ExternalInput").ap()
    gnw_ap = nc.dram_tensor("gnw", [C], F32, kind="ExternalInput").ap()
    gnb_ap = nc.dram_tensor("gnb", [C], F32, kind="ExternalInput").ap()
    gmat_ap = nc.dram_tensor("gmat", [128, CT, GROUPS], F32, kind="ExternalInput").ap()
    bmat_ap = nc.dram_tensor("bmat", [GROUPS, CT, 128], F32, kind="ExternalInput").ap()
    out_ap = nc.dram_tensor("out", [C, NH], F32, kind="ExternalOutput").ap()

    xf_v = xf_ap.rearrange("(t p) n -> p t n", p=128)       # [128, 2, 4096]
    out_v = out_ap.rearrange("(t p) n -> p t n", p=128)     # [128, 2, 2048]

    from contextlib import nullcontext

    with tile.TileContext(nc) as tc:
        rep_ctx = tc.For_i(0, reps, 1) if reps > 1 else nullcontext()
        with (
            rep_ctx,
            tc.tile_pool(name="consts", bufs=1) as consts,
            tc.tile_pool(name="big", bufs=1) as big,
            tc.tile_pool(name="xn_pool", bufs=3) as xn_pool,
            tc.tile_pool(name="p_pool", bufs=6) as p_pool,
            tc.tile_pool(name="att_pool", bufs=2) as att_pool,
            tc.tile_pool(name="epi_pool", bufs=3) as epi_pool,
            tc.tile_pool(name="stat_pool", bufs=1) as stat_pool,
        ):
            # ---------------- constants / weights ----------------
            w_stage = consts.tile([128, CT, 3 * C], F32)
            nc.sync.dma_start(w_stage[:], wqkvT_ap.rearrange("(t p) o -> p t o", p=128))
            wT = consts.tile([128, CT, 3 * C], F32R)
            nc.vector.tensor_copy(out=wT[:], in_=w_stage[:])

            wp_stage = consts.tile([128, CT, C], F32)
            nc.sync.dma_start(wp_stage[:], wpT_ap.rearrange("(t p) o -> p t o", p=128))
            wpT = consts.tile([128, CT, C], F32R)
            nc.vector.tensor_copy(out=wpT[:], in_=wp_stage[:])

            bq = consts.tile([128, CT], F32)
            nc.sync.dma_start(bq[:], bqkv_ap[0:C].rearrange("(t p) -> p t", p=128))
            bk = consts.tile([128, CT], F32)
            nc.sync.dma_start(bk[:], bqkv_ap[C:2 * C].rearrange("(t p) -> p t", p=128))
            cb_sb = consts.tile([128, CT], F32)
            nc.sync.dma_start(cb_sb[:], cb_ap.rearrange("(t p) -> p t", p=128))
            gnw_sb = consts.tile([128, CT], F32)
            nc.sync.dma_start(gnw_sb[:], gnw_ap.rearrange("(t p) -> p t", p=128))
            gnb_sb = consts.tile([128, CT], F32)
            nc.sync.dma_start(gnb_sb[:], gnb_ap.rearrange("(t p) -> p t", p=128))
            gmat = consts.tile([128, CT, GROUPS], F32)
            nc.sync.dma_start(gmat[:], gmat_ap)
            bmat = consts.tile([GROUPS, CT, 128], F32)
            nc.sync.dma_start(bmat[:], bmat_ap)

            ones_stage = consts.tile([128, 1], F32)
            nc.vector.memset(ones_stage[:], 1.0)
            ones_col = consts.tile([128, 1], F32R)      # lhsT for denominator
            nc.vector.tensor_copy(out=ones_col[:], in_=ones_stage[:])
            ones_row_stage = consts.tile([1, 128], F32)
            nc.vector.memset(ones_row_stage[:], 1.0)
            ones_row = consts.tile([1, 128], F32R)      # lhsT for recip broadcast
            nc.vector.tensor_copy(out=ones_row[:], in_=ones_row_stage[:])

            # ---------------- load x ----------------
            xf = big.tile([128, CT, N], F32)
            for ct in range(CT):
                nc.sync.dma_start(xf[:, ct, :], xf_v[:, ct, :])

            # ---------------- group norm stats ----------------
            stats = stat_pool.tile([128, CT, NCH, nc.vector.BN_STATS_DIM], F32)
            for ct in range(CT):
                for ch in range(NCH):
                    sl = slice(ch * CH, (ch + 1) * CH)
                    nc.vector.bn_stats(
                        out=stats[:, ct, ch, :], in_=xf[:, ct, sl]
                    )
            mv = stat_pool.tile([128, CT, 2], F32)
            for ct in range(CT):
                nc.vector.bn_aggr(out=mv[:, ct, :], in_=stats[:, ct, :, :])

            # rhs2[:, ct, :] = (mean_c, E[x^2]_c) per channel
            rhs2 = stat_pool.tile([128, CT, 2], F32)
            for ct in range(CT):
                nc.vector.tensor_copy(out=rhs2[:, ct, 0:1], in_=mv[:, ct, 0:1])
                nc.vector.scalar_tensor_tensor(
                    out=rhs2[:, ct, 1:2],
                    in0=mv[:, ct, 0:1],
                    scalar=mv[:, ct, 0:1],
                    in1=mv[:, ct, 1:2],
                    op0=mybir.AluOpType.mult,
                    op1=mybir.AluOpType.add,
                )

            with tc.tile_pool(name="psum_stat", bufs=1, space="PSUM") as psum_stat:
                ps_g = psum_stat.tile([GROUPS, 2], F32)
                for ct in range(CT):
                    nc.tensor.matmul(
                        ps_g[:], gmat[:, ct, :], rhs2[:, ct, :],
                        start=(ct == 0), stop=(ct == CT - 1),
                    )
                # per-group (mean, E2) then rstd
                grp = stat_pool.tile([GROUPS, 2], F32)
                nc.vector.tensor_scalar_mul(grp[:], ps_g[:], 1.0 / GSIZE)
                negvar = stat_pool.tile([GROUPS, 1], F32)
                # mu^2 - E2  (negated variance)
                nc.vector.scalar_tensor_tensor(
                    out=negvar[:],
                    in0=grp[:, 0:1],
                    scalar=grp[:, 0:1],
                    in1=grp[:, 1:2],
                    op0=mybir.AluOpType.mult,
                    op1=mybir.AluOpType.subtract,
                )
                grp2 = stat_pool.tile([GROUPS, 2], F32)
                nc.vector.tensor_copy(out=grp2[:, 0:1], in_=grp[:, 0:1])
                eps_t = stat_pool.tile([GROUPS, 1], F32)
                nc.vector.memset(eps_t[:], EPS)
                sq = stat_pool.tile([GROUPS, 1], F32)
                # sqrt(eps - negvar) = sqrt(var + eps)
                nc.scalar.activation(
                    out=sq[:], in_=negvar[:],
                    func=mybir.ActivationFunctionType.Sqrt,
                    bias=eps_t[:], scale=-1.0,
                )
                nc.vector.reciprocal(out=grp2[:, 1:2], in_=sq[:])

                # broadcast (mean_g, rstd_g) to channels via B matmul
                s_sb = stat_pool.tile([128, CT], F32)
                t_sb = stat_pool.tile([128, CT], F32)
                for ct in range(CT):
                    ps_bc = psum_stat.tile([128, 2], F32, tag="ps_bc")
                    nc.tensor.matmul(
                        ps_bc[:], bmat[:, ct, :], grp2[:], start=True, stop=True
                    )
                    # s = rstd * gn_w ; t = gn_b - mean * s
                    nc.vector.tensor_tensor(
                        out=s_sb[:, ct:ct + 1], in0=ps_bc[:, 1:2],
                        in1=gnw_sb[:, ct:ct + 1], op=mybir.AluOpType.mult,
                    )
                    tmp = stat_pool.tile([128, CT], F32, tag="tmp_mu_s")
                    nc.vector.tensor_tensor(
                        out=tmp[:, ct:ct + 1], in0=ps_bc[:, 0:1],
                        in1=s_sb[:, ct:ct + 1], op=mybir.AluOpType.mult,
                    )
                    nc.vector.tensor_tensor(
                        out=t_sb[:, ct:ct + 1], in0=gnb_sb[:, ct:ct + 1],
                        in1=tmp[:, ct:ct + 1], op=mybir.AluOpType.subtract,
                    )

            # ---------------- projections: K, Q, V^T ----------------
            K_sb = big.tile([128, CT, N], F32R)
            Q_sb = big.tile([128, CT, NH], F32R)
            VT_sb = big.tile([128, KT, C], F32R)

            with tc.tile_pool(name="psum_pre", bufs=4, space="PSUM") as psum_pre:
                for ch in range(NCH):
                    sl = slice(ch * CH, (ch + 1) * CH)
                    xn_t = xn_pool.tile([128, CT, CH], F32R, tag="xn")
                    for ct in range(CT):
                        nc.vector.tensor_scalar(
                            out=xn_t[:, ct, :], in0=xf[:, ct, sl],
                            scalar1=s_sb[:, ct:ct + 1], scalar2=t_sb[:, ct:ct + 1],
                            op0=mybir.AluOpType.mult, op1=mybir.AluOpType.add,
                        )
                    # K (and Q for first half chunks)
                    for ot in range(CT):
                        ps_k = psum_pre.tile([128, CH], F32, tag="ps_kq")
                        for ct in range(CT):
                            nc.tensor.matmul(
                                ps_k[:],
                                wT[:, ct, C + ot * 128:C + (ot + 1) * 128],
                                xn_t[:, ct, :],
                                start=(ct == 0), stop=(ct == CT - 1),
                            )
                        nc.vector.tensor_scalar(
                            out=K_sb[:, ot, sl], in0=ps_k[:],
                            scalar1=bk[:, ot:ot + 1], scalar2=None,
                            op0=mybir.AluOpType.add,
                        )
                        if ch < QC:
                            ps_q = psum_pre.tile([128, CH], F32, tag="ps_kq")
                            for ct in range(CT):
                                nc.tensor.matmul(
                                    ps_q[:],
                                    wT[:, ct, ot * 128:(ot + 1) * 128],
                                    xn_t[:, ct, :],
                                    start=(ct == 0), stop=(ct == CT - 1),
                                )
                            nc.vector.tensor_scalar(
                                out=Q_sb[:, ot, sl], in0=ps_q[:],
                                scalar1=bq[:, ot:ot + 1], scalar2=None,
                                op0=mybir.AluOpType.add,
                            )
                    # V^T: out[pix, c] accumulated over input-channel tiles
                    for pt in range(4):
                        ps_v = psum_pre.tile([128, C], F32, tag="ps_v")
                        for ct in range(CT):
                            nc.tensor.matmul(
                                ps_v[:],
                                xn_t[:, ct, pt * 128:(pt + 1) * 128],
                                wT[:, ct, 2 * C:3 * C],
                                start=(ct == 0), stop=(ct == CT - 1),
                            )
                        nc.vector.tensor_copy(
                            out=VT_sb[:, ch * 4 + pt, :], in_=ps_v[:]
                        )

            # ---------------- attention ----------------
            # Software-pipelined: the PV matmuls for key-tile kt are emitted D
            # key-tiles behind the scores/exp for kt, so the PE never stalls
            # on the ScalarE exp latency. The softmax denominator accumulates
            # on VectorE (acc_d += p_t) and needs only one ones-matmul per
            # q-chunk instead of one per key-tile.
            D = 4
            with (
                tc.tile_pool(name="psum_s", bufs=D + 1, space="PSUM") as psum_s,
                tc.tile_pool(name="psum_o", bufs=1, space="PSUM") as psum_o,
                tc.tile_pool(name="psum_d", bufs=1, space="PSUM") as psum_d,
            ):
                for qc in range(QC):
                    qsl = slice(qc * CH, (qc + 1) * CH)
                    ps_o = psum_o.tile([128, CT, CH], F32, tag="ps_o")
                    acc_d = epi_pool.tile([128, CH], F32R, tag="acc_d")
                    p_tiles = [None] * KT

                    def emit_scores(kt):
                        ps_s = psum_s.tile([128, CH], F32, tag="ps_s")
                        ksl = slice(kt * 128, (kt + 1) * 128)
                        for ct in range(CT):
                            nc.tensor.matmul(
                                ps_s[:], K_sb[:, ct, ksl], Q_sb[:, ct, qsl],
                                start=(ct == 0), stop=(ct == CT - 1),
                            )
                        p_t = p_pool.tile([128, CH], F32R, tag="p")
                        nc.scalar.activation(
                            out=p_t[:], in_=ps_s[:],
                            func=mybir.ActivationFunctionType.Exp,
                            scale=float(SCALE),
                        )
                        p_tiles[kt] = p_t

                    def emit_pv(kt):
                        p_t = p_tiles[kt]
                        for ot in range(CT):
                            nc.tensor.matmul(
                                ps_o[:, ot, :],
                                VT_sb[:, kt, ot * 128:(ot + 1) * 128],
                                p_t[:],
                                start=(kt == 0), stop=(kt == KT - 1),
                            )
                        with nc.allow_low_precision(
                            reason="f32r accumulation of softmax denominator; "
                                   "~1e-4 relative effect"
                        ):
                            if kt == 0:
                                nc.vector.tensor_copy(out=acc_d[:], in_=p_t[:])
                            else:
                                nc.vector.tensor_add(acc_d[:], acc_d[:], p_t[:])
                        p_tiles[kt] = None

                    for kt in range(KT):
                        emit_scores(kt)
                        if kt >= D:
                            emit_pv(kt - D)
                    for kt in range(KT - D, KT):
                        emit_pv(kt)
                    # softmax denominator -> broadcast reciprocal
                    ps_d = psum_d.tile([1, CH], F32, tag="ps_d")
                    nc.tensor.matmul(ps_d[:], ones_col[:], acc_d[:],
                                     start=True, stop=True)
                    r_sb = epi_pool.tile([1, CH], F32R, tag="r_sb")
                    with nc.allow_low_precision(
                        reason="f32r-rounded reciprocal of softmax denominator; "
                               "~6e-5 relative effect on output"
                    ):
                        nc.vector.reciprocal(out=r_sb[:], in_=ps_d[:])
                    ps_r = psum_s.tile([128, CH], F32, tag="ps_s")
                    nc.tensor.matmul(ps_r[:], ones_row[:], r_sb[:],
                                     start=True, stop=True)
                    r_bc = epi_pool.tile([128, CH], F32, tag="r_bc")
                    nc.vector.tensor_copy(out=r_bc[:], in_=ps_r[:])
                    # attention output (unnormalized) -> sbuf for proj matmul
                    att_t = att_pool.tile([128, CT, CH], F32R, tag="att")
                    for ot in range(CT):
                        nc.vector.tensor_copy(out=att_t[:, ot, :], in_=ps_o[:, ot, :])
                    for ot in range(CT):
                        ps_p = psum_s.tile([128, CH], F32, tag="ps_s")
                        for ci in range(CT):
                            nc.tensor.matmul(
                                ps_p[:],
                                wpT[:, ci, ot * 128:(ot + 1) * 128],
                                att_t[:, ci, :],
                                start=(ci == 0), stop=(ci == CT - 1),
                            )
                        f1 = epi_pool.tile([128, CH], F32, tag="f1")
                        nc.vector.tensor_tensor(
                            out=f1[:], in0=ps_p[:], in1=r_bc[:],
                            op=mybir.AluOpType.mult,
                        )
                        f2 = epi_pool.tile([128, CH], F32, tag="f2")
                        nc.vector.scalar_tensor_tensor(
                            out=f2[:], in0=f1[:], scalar=cb_sb[:, ot:ot + 1],
                            in1=xf[:, ot, qsl],
                            op0=mybir.AluOpType.add, op1=mybir.AluOpType.add,
                        )
                        nc.sync.dma_start(out_v[:, ot, qsl], f2[:])


def _host_consts():
    gmat = np.zeros((128, CT, GROUPS), np.float32)
    bmat = np.zeros((GROUPS, CT, 128), np.float32)
    for t in range(CT):
        for p in range(128):
            g = (t * 128 + p) // GSIZE
            gmat[p, t, g] = 1.0
            bmat[g, t, p] = 1.0
    return gmat, bmat


_CACHE = {}


def _get_nc(reps=1):
    key = ("nc", reps)
    if key not in _CACHE:
        nc = bacc.Bacc("TRN2", target_bir_lowering=False, debug=False,
                       num_devices=N_CORES)
        _trace_kernel(nc, reps=reps)
        nc.compile()
        _CACHE[key] = nc
    return _CACHE[key]


def _make_in_maps(x, gn_w, gn_b, qkv_w, qkv_b, proj_w, proj_b):
    gmat, bmat = _host_consts()
    wqkvT = np.ascontiguousarray(qkv_w.T.astype(np.float32))
    wpT = np.ascontiguousarray(proj_w.T.astype(np.float32))
    # v-bias folds through softmax (weights sum to 1) and the projection:
    # out += proj_w @ bv + proj_b, a weight-only constant.
    cb = (proj_w.astype(np.float64) @ qkv_b[2 * C:3 * C].astype(np.float64)
          + proj_b.astype(np.float64)).astype(np.float32)
    in_maps = []
    for core in range(N_CORES):
        b, h = core // 2, core % 2
        xr = np.ascontiguousarray(x[b].reshape(C, N).astype(np.float32))
        if h == 1:
            xr = np.ascontiguousarray(
                np.concatenate([xr[:, NH:], xr[:, :NH]], axis=1)
            )
        in_maps.append({
            "xf": xr,
            "wqkvT": wqkvT,
            "bqkv": qkv_b.astype(np.float32),
            "wpT": wpT,
            "cb": cb,
            "gnw": gn_w.astype(np.float32),
            "gnb": gn_b.astype(np.float32),
            "gmat": gmat,
            "bmat": bmat,
        })
    return in_maps


def _get_runner(reps=1):
    """Build (once) a reusable jitted PJRT runner over the 8 axon cores."""
    key = ("runner", reps)
    if key in _CACHE:
        return _CACHE[key]
    import jax
    from jax.sharding import Mesh, PartitionSpec
    from jax.experimental.shard_map import shard_map
    from concourse import bass2jax
    from concourse.bass2jax import _bass_exec_p, partition_id_tensor

    nc = _get_nc(reps)
    bass2jax.install_neuronx_cc_hook()

    partition_name = nc.partition_id_tensor.name if nc.partition_id_tensor else None
    in_names, out_names, out_avals, zero_outs = [], [], [], []
    for alloc in nc.m.functions[0].allocations:
        if not isinstance(alloc, mybir.MemoryLocationSet):
            continue
        name = alloc.memorylocations[0].name
        if alloc.kind == "ExternalInput":
            if name != partition_name:
                in_names.append(name)
        elif alloc.kind == "ExternalOutput":
            out_names.append(name)
            shape = tuple(alloc.tensor_shape)
            dtype = mybir.dt.np(alloc.dtype)
            out_avals.append(jax.core.ShapedArray(shape, dtype))
            zero_outs.append(np.zeros(shape, dtype))
    n_params = len(in_names)
    n_outs = len(out_avals)
    all_in_names = list(in_names) + list(out_names)
    if partition_name is not None:
        all_in_names.append(partition_name)

    def _body(*args):
        operands = list(args)
        if partition_name is not None:
            operands.append(partition_id_tensor())
        outs = _bass_exec_p.bind(
            *operands,
            out_avals=tuple(out_avals),
            in_names=tuple(all_in_names),
            out_names=tuple(out_names),
            lowering_input_output_aliases=(),
            sim_require_finite=True,
            sim_require_nnan=True,
            nc=nc,
        )
        return tuple(outs)

    devices = jax.devices()[:N_CORES]
    mesh = Mesh(np.asarray(devices), ("core",))
    in_specs = (PartitionSpec("core"),) * (n_params + n_outs)
    out_specs = (PartitionSpec("core"),) * n_outs
    donate = tuple(range(n_params, n_params + n_outs))
    sharded = jax.jit(
        shard_map(_body, mesh=mesh, in_specs=in_specs, out_specs=out_specs,
                  check_rep=False),
        donate_argnums=donate, keep_unused=True,
    )

    in_shardings = [jax.sharding.NamedSharding(mesh, PartitionSpec("core"))
                    for _ in range(n_params + n_outs)]

    def pack(in_maps):
        """Host->device once; returns device-resident positional inputs."""
        concat_in = [
            np.concatenate([np.asarray(in_maps[c][nm]) for c in range(N_CORES)],
                           axis=0)
            for nm in in_names
        ]
        return [jax.device_put(a, s) for a, s in zip(concat_in, in_shardings)]

    def fresh_zeros():
        return [
            jax.device_put(
                np.zeros((N_CORES * z.shape[0], *z.shape[1:]), z.dtype), s)
            for z, s in zip(zero_outs, in_shardings[n_params:])
        ]

    def exec_packed(dev_in, block=True):
        out_arrs = sharded(*dev_in, *fresh_zeros())
        if block:
            for o in out_arrs:
                o.block_until_ready()
        return out_arrs

    def run(in_maps):
        out_arrs = exec_packed(pack(in_maps))
        return [
            {
                nm: np.asarray(out_arrs[i]).reshape(N_CORES, *out_avals[i].shape)[c]
                for i, nm in enumerate(out_names)
            }
            for c in range(N_CORES)
        ]

    run.pack = pack
    run.exec_packed = exec_packed
    run.fresh_zeros = fresh_zeros
    _CACHE[key] = run
    return run


def kernel(x, gn_w, gn_b, qkv_w, qkv_b, proj_w, proj_b):
    run = _get_runner()
    in_maps = _make_in_maps(x, gn_w, gn_b, qkv_w, qkv_b, proj_w, proj_b)
    results = run(in_maps)
    y = np.empty((B, C, N), np.float32)
    for core in range(N_CORES):
        b, h = core // 2, core % 2
        y[b, :, h * NH:(h + 1) * NH] = results[core]["out"]
    return y.reshape(B, C, HH, WW)


# revision 25
# speedup vs baseline: 1.1293x; 1.1293x over previous
"""AttentionBlock (GroupNorm + single-head self-attention + proj + residual)
for x[4, 256, 64, 64] on 8 Trainium2 NeuronCores.

Sharding: data-parallel over batch (4) x sequence-parallel over the q/hw dim
(2 halves) = 8 shards, one per core. Each core receives its batch's full
x (rolled so its q-half sits at columns 0:2048), computes GroupNorm + full
K/V + its half of Q, then attention over all 4096 keys for its 2048 queries,
projection and residual. No collectives; host assembles the 8 output shards.

All matmuls run in float32r (full PE rate at N>=256, ~1e-4 relative error).
Attention is computed in "scores-transposed" layout sT[k, q] so that the
P@V contraction consumes exp(sT) directly without any transposes. Softmax
runs without max-subtraction (scores/16 ~ N(0,1), far from overflow); the
denominator accumulates on VectorE and one ones-matmul reduces it across
partitions, with the reciprocal broadcast back via a K=1 matmul.
"""

import numpy as np

import concourse.bass as bass
import concourse.bacc as bacc
import concourse.tile as tile
from concourse import mybir

F32 = mybir.dt.float32
F32R = mybir.dt.float32r

N_CORES = 8
B, C, HH, WW = 4, 256, 64, 64
N = HH * WW            # 4096 pixels (keys)
NH = N // 2            # 2048 queries per core
CT = C // 128          # 2 channel tiles
CH = 512               # free-dim chunk for projections / q-chunks
NCH = N // CH          # 8 chunks over all pixels
QC = NH // CH          # 4 q-chunks
KT = N // 128          # 32 key tiles
GROUPS = 16
GSIZE = C // GROUPS    # 16 channels per group
EPS = 1e-5
SCALE = 1.0 / np.sqrt(np.float32(C))   # 1/16


def _trace_kernel(nc, reps=1):
    xf_ap = nc.dram_tensor("xf", [C, N], F32, kind="ExternalInput").ap()
    wqkvT_ap = nc.dram_tensor("wqkvT", [C, 3 * C], F32, kind="ExternalInput").ap()
    bqkv_ap = nc.dram_tensor("bqkv", [3 * C], F32, kind="ExternalInput").ap()
    wpT_ap = nc.dram_tensor("wpT", [C, C], F32, kind="ExternalInput").ap()
    cb_ap = nc.dram_tensor("cb", [C], F32, kind="ExternalInput").ap()
    gnw_ap = nc.dram_tensor("gnw", [C], F32, kind="ExternalInput").ap()
    gnb_ap = nc.dram_tensor("gnb", [C], F32, kind="ExternalInput").ap()
    gmat_ap = nc.dram_tensor("gmat", [128, CT, GROUPS], F32, kind="ExternalInput").ap()
    bmat_ap = nc.dram_tensor("bmat", [GROUPS, CT, 128], F32, kind="ExternalInput").ap()
    out_ap = nc.dram_tensor("out", [C, NH], F32, kind="ExternalOutput").ap()

    xf_v = xf_ap.rearrange("(t p) n -> p t n", p=128)       # [128, 2, 4096]
    out_v = out_ap.rearrange("(t p) n -> p t n", p=128)     # [128, 2, 2048]

    from contextlib import nullcontext

    with tile.TileContext(nc) as tc:
        # hint_engines: the loop body far exceeds one IRAM block per engine,
        # so arm the branch prefetcher to avoid a ~4us I$-miss per back-edge
        # (timing builds only; reps=1 has no loop).
        rep_ctx = (
            tc.For_i(0, reps, 1, hint_engines=(
                mybir.EngineType.PE, mybir.EngineType.Activation,
                mybir.EngineType.DVE, mybir.EngineType.SP,
                mybir.EngineType.Pool,
            ))
            if reps > 1 else nullcontext()
        )
        with (
            rep_ctx,
            tc.tile_pool(name="consts", bufs=1) as consts,
            tc.tile_pool(name="big", bufs=1) as big,
            tc.tile_pool(name="xn_pool", bufs=3) as xn_pool,
            tc.tile_pool(name="p_pool", bufs=6) as p_pool,
            tc.tile_pool(name="att_pool", bufs=2) as att_pool,
            tc.tile_pool(name="epi_pool", bufs=3) as epi_pool,
            tc.tile_pool(name="stat_pool", bufs=1) as stat_pool,
        ):
            # ---------------- constants / weights ----------------
            w_stage = consts.tile([128, CT, 3 * C], F32)
            nc.sync.dma_start(w_stage[:], wqkvT_ap.rearrange("(t p) o -> p t o", p=128))
            wT = consts.tile([128, CT, 3 * C], F32R)
            nc.vector.tensor_copy(out=wT[:], in_=w_stage[:])

            wp_stage = consts.tile([128, CT, C], F32)
            nc.sync.dma_start(wp_stage[:], wpT_ap.rearrange("(t p) o -> p t o", p=128))
            wpT = consts.tile([128, CT, C], F32R)
            nc.vector.tensor_copy(out=wpT[:], in_=wp_stage[:])

            bq = consts.tile([128, CT], F32)
            nc.sync.dma_start(bq[:], bqkv_ap[0:C].rearrange("(t p) -> p t", p=128))
            bk = consts.tile([128, CT], F32)
            nc.sync.dma_start(bk[:], bqkv_ap[C:2 * C].rearrange("(t p) -> p t", p=128))
            cb_sb = consts.tile([128, CT], F32)
            nc.sync.dma_start(cb_sb[:], cb_ap.rearrange("(t p) -> p t", p=128))
            gnw_sb = consts.tile([128, CT], F32)
            nc.sync.dma_start(gnw_sb[:], gnw_ap.rearrange("(t p) -> p t", p=128))
            gnb_sb = consts.tile([128, CT], F32)
            nc.sync.dma_start(gnb_sb[:], gnb_ap.rearrange("(t p) -> p t", p=128))
            gmat = consts.tile([128, CT, GROUPS], F32)
            nc.sync.dma_start(gmat[:], gmat_ap)
            bmat = consts.tile([GROUPS, CT, 128], F32)
            nc.sync.dma_start(bmat[:], bmat_ap)

            ones_stage = consts.tile([128, 1], F32)
            nc.vector.memset(ones_stage[:], 1.0)
            ones_col = consts.tile([128, 1], F32R)      # lhsT for denominator
            nc.vector.tensor_copy(out=ones_col[:], in_=ones_stage[:])
            ones_row_stage = consts.tile([1, 128], F32)
            nc.vector.memset(ones_row_stage[:], 1.0)
            ones_row = consts.tile([1, 128], F32R)      # lhsT for recip broadcast
            nc.vector.tensor_copy(out=ones_row[:], in_=ones_row_stage[:])

            # ---------------- load x ----------------
            xf = big.tile([128, CT, N], F32)
            for ct in range(CT):
                nc.sync.dma_start(xf[:, ct, :], xf_v[:, ct, :])

            # ---------------- group norm stats ----------------
            stats = stat_pool.tile([128, CT, NCH, nc.vector.BN_STATS_DIM], F32)
            for ct in range(CT):
                for ch in range(NCH):
                    sl = slice(ch * CH, (ch + 1) * CH)
                    nc.vector.bn_stats(
                        out=stats[:, ct, ch, :], in_=xf[:, ct, sl]
                    )
            mv = stat_pool.tile([128, CT, 2], F32)
            for ct in range(CT):
                nc.vector.bn_aggr(out=mv[:, ct, :], in_=stats[:, ct, :, :])

            # rhs2[:, ct, :] = (mean_c, E[x^2]_c) per channel
            rhs2 = stat_pool.tile([128, CT, 2], F32)
            for ct in range(CT):
                nc.vector.tensor_copy(out=rhs2[:, ct, 0:1], in_=mv[:, ct, 0:1])
                nc.vector.scalar_tensor_tensor(
                    out=rhs2[:, ct, 1:2],
                    in0=mv[:, ct, 0:1],
                    scalar=mv[:, ct, 0:1],
                    in1=mv[:, ct, 1:2],
                    op0=mybir.AluOpType.mult,
                    op1=mybir.AluOpType.add,
                )

            with tc.tile_pool(name="psum_stat", bufs=1, space="PSUM") as psum_stat:
                ps_g = psum_stat.tile([GROUPS, 2], F32)
                for ct in range(CT):
                    nc.tensor.matmul(
                        ps_g[:], gmat[:, ct, :], rhs2[:, ct, :],
                        start=(ct == 0), stop=(ct == CT - 1),
                    )
                # per-group (mean, E2) then rstd
                grp = stat_pool.tile([GROUPS, 2], F32)
                nc.vector.tensor_scalar_mul(grp[:], ps_g[:], 1.0 / GSIZE)
                negvar = stat_pool.tile([GROUPS, 1], F32)
                # mu^2 - E2  (negated variance)
                nc.vector.scalar_tensor_tensor(
                    out=negvar[:],
                    in0=grp[:, 0:1],
                    scalar=grp[:, 0:1],
                    in1=grp[:, 1:2],
                    op0=mybir.AluOpType.mult,
                    op1=mybir.AluOpType.subtract,
                )
                grp2 = stat_pool.tile([GROUPS, 2], F32)
                nc.vector.tensor_copy(out=grp2[:, 0:1], in_=grp[:, 0:1])
                eps_t = stat_pool.tile([GROUPS, 1], F32)
                nc.vector.memset(eps_t[:], EPS)
                sq = stat_pool.tile([GROUPS, 1], F32)
                # sqrt(eps - negvar) = sqrt(var + eps)
                nc.scalar.activation(
                    out=sq[:], in_=negvar[:],
                    func=mybir.ActivationFunctionType.Sqrt,
                    bias=eps_t[:], scale=-1.0,
                )
                nc.vector.reciprocal(out=grp2[:, 1:2], in_=sq[:])

                # broadcast (mean_g, rstd_g) to channels via B matmul
                s_sb = stat_pool.tile([128, CT], F32)
                t_sb = stat_pool.tile([128, CT], F32)
                for ct in range(CT):
                    ps_bc = psum_stat.tile([128, 2], F32, tag="ps_bc")
                    nc.tensor.matmul(
                        ps_bc[:], bmat[:, ct, :], grp2[:], start=True, stop=True
                    )
                    # s = rstd * gn_w ; t = gn_b - mean * s
                    nc.vector.tensor_tensor(
                        out=s_sb[:, ct:ct + 1], in0=ps_bc[:, 1:2],
                        in1=gnw_sb[:, ct:ct + 1], op=mybir.AluOpType.mult,
                    )
                    tmp = stat_pool.tile([128, CT], F32, tag="tmp_mu_s")
                    nc.vector.tensor_tensor(
                        out=tmp[:, ct:ct + 1], in0=ps_bc[:, 0:1],
                        in1=s_sb[:, ct:ct + 1], op=mybir.AluOpType.mult,
                    )
                    nc.vector.tensor_tensor(
                        out=t_sb[:, ct:ct + 1], in0=gnb_sb[:, ct:ct + 1],
                        in1=tmp[:, ct:ct + 1], op=mybir.AluOpType.subtract,
                    )

            # ---------------- projections: K, Q, V^T ----------------
            K_sb = big.tile([128, CT, N], F32R)
            Q_sb = big.tile([128, CT, NH], F32R)
            VT_sb = big.tile([128, KT, C], F32R)

            with tc.tile_pool(name="psum_pre", bufs=4, space="PSUM") as psum_pre:
                for ch in range(NCH):
                    sl = slice(ch * CH, (ch + 1) * CH)
                    xn_t = xn_pool.tile([128, CT, CH], F32R, tag="xn")
                    for ct in range(CT):
                        nc.vector.tensor_scalar(
                            out=xn_t[:, ct, :], in0=xf[:, ct, sl],
                            scalar1=s_sb[:, ct:ct + 1], scalar2=t_sb[:, ct:ct + 1],
                            op0=mybir.AluOpType.mult, op1=mybir.AluOpType.add,
                        )
                    # K (and Q for first half chunks)
                    for ot in range(CT):
                        ps_k = psum_pre.tile([128, CH], F32, tag="ps_kq")
                        for ct in range(CT):
                            nc.tensor.matmul(
                                ps_k[:],
                                wT[:, ct, C + ot * 128:C + (ot + 1) * 128],
                                xn_t[:, ct, :],
                                start=(ct == 0), stop=(ct == CT - 1),
                            )
                        nc.vector.tensor_scalar(
                            out=K_sb[:, ot, sl], in0=ps_k[:],
                            scalar1=bk[:, ot:ot + 1], scalar2=None,
                            op0=mybir.AluOpType.add,
                        )
                        if ch < QC:
                            ps_q = psum_pre.tile([128, CH], F32, tag="ps_kq")
                            for ct in range(CT):
                                nc.tensor.matmul(
                                    ps_q[:],
                                    wT[:, ct, ot * 128:(ot + 1) * 128],
                                    xn_t[:, ct, :],
                                    start=(ct == 0), stop=(ct == CT - 1),
                                )
                            nc.vector.tensor_scalar(
                                out=Q_sb[:, ot, sl], in0=ps_q[:],
                                scalar1=bq[:, ot:ot + 1], scalar2=None,
                                op0=mybir.AluOpType.add,
                            )
                    # V^T: out[pix, c] accumulated over input-channel tiles
                    for pt in range(4):
                        ps_v = psum_pre.tile([128, C], F32, tag="ps_v")
                        for ct in range(CT):
                            nc.tensor.matmul(
                                ps_v[:],
                                xn_t[:, ct, pt * 128:(pt + 1) * 128],
                                wT[:, ct, 2 * C:3 * C],
                                start=(ct == 0), stop=(ct == CT - 1),
                            )
                        nc.vector.tensor_copy(
                            out=VT_sb[:, ch * 4 + pt, :], in_=ps_v[:]
                        )

            # ---------------- attention ----------------
            # Software-pipelined: the PV matmuls for key-tile kt are emitted D
            # key-tiles behind the scores/exp for kt, so the PE never stalls
            # on the ScalarE exp latency. The softmax denominator accumulates
            # on VectorE (acc_d += p_t) and needs only one ones-matmul per
            # q-chunk instead of one per key-tile.
            D = 4
            with (
                tc.tile_pool(name="psum_s", bufs=D + 1, space="PSUM") as psum_s,
                tc.tile_pool(name="psum_o", bufs=1, space="PSUM") as psum_o,
                tc.tile_pool(name="psum_d", bufs=1, space="PSUM") as psum_d,
            ):
                for qc in range(QC):
                    qsl = slice(qc * CH, (qc + 1) * CH)
                    ps_o = psum_o.tile([128, CT, CH], F32, tag="ps_o")
                    acc_d = epi_pool.tile([128, CH], F32R, tag="acc_d")
                    p_tiles = [None] * KT

                    def emit_scores(kt):
                        ps_s = psum_s.tile([128, CH], F32, tag="ps_s")
                        ksl = slice(kt * 128, (kt + 1) * 128)
                        for ct in range(CT):
                            nc.tensor.matmul(
                                ps_s[:], K_sb[:, ct, ksl], Q_sb[:, ct, qsl],
                                start=(ct == 0), stop=(ct == CT - 1),
                            )
                        p_t = p_pool.tile([128, CH], F32R, tag="p")
                        nc.scalar.activation(
                            out=p_t[:], in_=ps_s[:],
                            func=mybir.ActivationFunctionType.Exp,
                            scale=float(SCALE),
                        )
                        p_tiles[kt] = p_t

                    def emit_pv(kt):
                        p_t = p_tiles[kt]
                        for ot in range(CT):
                            nc.tensor.matmul(
                                ps_o[:, ot, :],
                                VT_sb[:, kt, ot * 128:(ot + 1) * 128],
                                p_t[:],
                                start=(kt == 0), stop=(kt == KT - 1),
                            )
                        with nc.allow_low_precision(
                            reason="f32r accumulation of softmax denominator; "
                                   "~1e-4 relative effect"
                        ):
                            if kt == 0:
                                nc.vector.tensor_copy(out=acc_d[:], in_=p_t[:])
                            else:
                                nc.vector.tensor_add(acc_d[:], acc_d[:], p_t[:])
                        p_tiles[kt] = None

                    for kt in range(KT):
                        emit_scores(kt)
                        if kt >= D:
                            emit_pv(kt - D)
                    for kt in range(KT - D, KT):
                        emit_pv(kt)
                    # softmax denominator -> broadcast reciprocal
                    ps_d = psum_d.tile([1, CH], F32, tag="ps_d")
                    nc.tensor.matmul(ps_d[:], ones_col[:], acc_d[:],
                                     start=True, stop=True)
                    r_sb = epi_pool.tile([1, CH], F32R, tag="r_sb")
                    with nc.allow_low_precision(
                        reason="f32r-rounded reciprocal of softmax denominator; "
                               "~6e-5 relative effect on output"
                    ):
                        nc.vector.reciprocal(out=r_sb[:], in_=ps_d[:])
                    ps_r = psum_s.tile([128, CH], F32, tag="ps_s")
                    nc.tensor.matmul(ps_r[:], ones_row[:], r_sb[:],
                                     start=True, stop=True)
                    r_bc = epi_pool.tile([128, CH], F32, tag="r_bc")
                    nc.vector.tensor_copy(out=r_bc[:], in_=ps_r[:])
                    # attention output (unnormalized) -> sbuf for proj matmul
                    att_t = att_pool.tile([128, CT, CH], F32R, tag="att")
                    for ot in range(CT):
                        nc.vector.tensor_copy(out=att_t[:, ot, :], in_=ps_o[:, ot, :])
                    for ot in range(CT):
                        ps_p = psum_s.tile([128, CH], F32, tag="ps_s")
                        for ci in range(CT):
                            nc.tensor.matmul(
                                ps_p[:],
                                wpT[:, ci, ot * 128:(ot + 1) * 128],
                                att_t[:, ci, :],
                                start=(ci == 0), stop=(ci == CT - 1),
                            )
                        f1 = epi_pool.tile([128, CH], F32, tag="f1")
                        nc.vector.tensor_tensor(
                            out=f1[:], in0=ps_p[:], in1=r_bc[:],
                            op=mybir.AluOpType.mult,
                        )
                        f2 = epi_pool.tile([128, CH], F32, tag="f2")
                        nc.vector.scalar_tensor_tensor(
                            out=f2[:], in0=f1[:], scalar=cb_sb[:, ot:ot + 1],
                            in1=xf[:, ot, qsl],
                            op0=mybir.AluOpType.add, op1=mybir.AluOpType.add,
                        )
                        nc.sync.dma_start(out_v[:, ot, qsl], f2[:])


def _host_consts():
    gmat = np.zeros((128, CT, GROUPS), np.float32)
    bmat = np.zeros((GROUPS, CT, 128), np.float32)
    for t in range(CT):
        for p in range(128):
            g = (t * 128 + p) // GSIZE
            gmat[p, t, g] = 1.0
            bmat[g, t, p] = 1.0
    return gmat, bmat


_CACHE = {}


def _get_nc(reps=1):
    key = ("nc", reps)
    if key not in _CACHE:
        nc = bacc.Bacc("TRN2", target_bir_lowering=False, debug=False,
                       num_devices=N_CORES)
        _trace_kernel(nc, reps=reps)
        nc.compile()
        _CACHE[key] = nc
    return _CACHE[key]


def _make_in_maps(x, gn_w, gn_b, qkv_w, qkv_b, proj_w, proj_b):
    gmat, bmat = _host_consts()
    wqkvT = np.ascontiguousarray(qkv_w.T.astype(np.float32))
    wpT = np.ascontiguousarray(proj_w.T.astype(np.float32))
    # v-bias folds through softmax (weights sum to 1) and the projection:
    # out += proj_w @ bv + proj_b, a weight-only constant.
    cb = (proj_w.astype(np.float64) @ qkv_b[2 * C:3 * C].astype(np.float64)
          + proj_b.astype(np.float64)).astype(np.float32)
    in_maps = []
    for core in range(N_CORES):
        b, h = core // 2, core % 2
        xr = np.ascontiguousarray(x[b].reshape(C, N).astype(np.float32))
        if h == 1:
            xr = np.ascontiguousarray(
                np.concatenate([xr[:, NH:], xr[:, :NH]], axis=1)
            )
        in_maps.append({
            "xf": xr,
            "wqkvT": wqkvT,
            "bqkv": qkv_b.astype(np.float32),
            "wpT": wpT,
            "cb": cb,
            "gnw": gn_w.astype(np.float32),
            "gnb": gn_b.astype(np.float32),
            "gmat": gmat,
            "bmat": bmat,
        })
    return in_maps


def _get_runner(reps=1):
    """Build (once) a reusable jitted PJRT runner over the 8 axon cores."""
    key = ("runner", reps)
    if key in _CACHE:
        return _CACHE[key]
    import jax
    from jax.sharding import Mesh, PartitionSpec
    from jax.experimental.shard_map import shard_map
    from concourse import bass2jax
    from concourse.bass2jax import _bass_exec_p, partition_id_tensor

    nc = _get_nc(reps)
    bass2jax.install_neuronx_cc_hook()

    partition_name = nc.partition_id_tensor.name if nc.partition_id_tensor else None
    in_names, out_names, out_avals, zero_outs = [], [], [], []
    for alloc in nc.m.functions[0].allocations:
        if not isinstance(alloc, mybir.MemoryLocationSet):
            continue
        name = alloc.memorylocations[0].name
        if alloc.kind == "ExternalInput":
            if name != partition_name:
                in_names.append(name)
        elif alloc.kind == "ExternalOutput":
            out_names.append(name)
            shape = tuple(alloc.tensor_shape)
            dtype = mybir.dt.np(alloc.dtype)
            out_avals.append(jax.core.ShapedArray(shape, dtype))
            zero_outs.append(np.zeros(shape, dtype))
    n_params = len(in_names)
    n_outs = len(out_avals)
    all_in_names = list(in_names) + list(out_names)
    if partition_name is not None:
        all_in_names.append(partition_name)

    def _body(*args):
        operands = list(args)
        if partition_name is not None:
            operands.append(partition_id_tensor())
        outs = _bass_exec_p.bind(
            *operands,
            out_avals=tuple(out_avals),
            in_names=tuple(all_in_names),
            out_names=tuple(out_names),
            lowering_input_output_aliases=(),
            sim_require_finite=True,
            sim_require_nnan=True,
            nc=nc,
        )
        return tuple(outs)

    devices = jax.devices()[:N_CORES]
    mesh = Mesh(np.asarray(devices), ("core",))
    in_specs = (PartitionSpec("core"),) * (n_params + n_outs)
    out_specs = (PartitionSpec("core"),) * n_outs
    donate = tuple(range(n_params, n_params + n_outs))
    sharded = jax.jit(
        shard_map(_body, mesh=mesh, in_specs=in_specs, out_specs=out_specs,
                  check_rep=False),
        donate_argnums=donate, keep_unused=True,
    )

    in_shardings = [jax.sharding.NamedSharding(mesh, PartitionSpec("core"))
                    for _ in range(n_params + n_outs)]

    def pack(in_maps):
        """Host->device once; returns device-resident positional inputs."""
        concat_in = [
            np.concatenate([np.asarray(in_maps[c][nm]) for c in range(N_CORES)],
                           axis=0)
            for nm in in_names
        ]
        return [jax.device_put(a, s) for a, s in zip(concat_in, in_shardings)]

    def fresh_zeros():
        return [
            jax.device_put(
                np.zeros((N_CORES * z.shape[0], *z.shape[1:]), z.dtype), s)
            for z, s in zip(zero_outs, in_shardings[n_params:])
        ]

    def exec_packed(dev_in, block=True):
        out_arrs = sharded(*dev_in, *fresh_zeros())
        if block:
            for o in out_arrs:
                o.block_until_ready()
        return out_arrs

    def run(in_maps):
        out_arrs = exec_packed(pack(in_maps))
        return [
            {
                nm: np.asarray(out_arrs[i]).reshape(N_CORES, *out_avals[i].shape)[c]
                for i, nm in enumerate(out_names)
            }
            for c in range(N_CORES)
        ]

    run.pack = pack
    run.exec_packed = exec_packed
    run.fresh_zeros = fresh_zeros
    _CACHE[key] = run
    return run


def kernel(x, gn_w, gn_b, qkv_w, qkv_b, proj_w, proj_b):
    run = _get_runner()
    in_maps = _make_in_maps(x, gn_w, gn_b, qkv_w, qkv_b, proj_w, proj_b)
    results = run(in_maps)
    y = np.empty((B, C, N), np.float32)
    for core in range(N_CORES):
        b, h = core // 2, core % 2
        y[b, :, h * NH:(h + 1) * NH] = results[core]["out"]
    return y.reshape(B, C, HH, WW)
